# revision 17
# baseline (speedup 1.0000x reference)
"""Trainium2 Bass kernel for a pre-norm transformer decoder layer.

Full inputs in, full output out. 8-way data-parallel over tokens
(batch 2 x 4 query-slices of 512). Each core redundantly computes K/V
for its batch's full 2048-token sequence, but chunk-by-chunk: the
LN1+K+V compute of key-chunk c+1 runs on TensorE inside the (scalar-
engine-bound) softmax-exp window of chunk c. Attention is chunk-outer/
head-inner with per-head PSUM partials spilled to SBUF accumulators on
GpSimd. Keys are processed in a per-core permuted order (own chunk
first) -- softmax is key-order invariant.

Shapes: x (2, 2048, 1024), 16 heads, dk=64, d_ff=2048, eps=1e-5.
"""
import threading

import numpy as np
import ml_dtypes

import concourse.mybir as mybir
import concourse.tile as tile
from concourse import bacc
from concourse.bass_utils import run_bass_kernel_spmd
from contextlib import ExitStack

F32 = mybir.dt.float32
BF16 = mybir.dt.bfloat16
AF = mybir.ActivationFunctionType
OP = mybir.AluOpType

B, S, D = 2, 2048, 1024
H, DK, FF = 16, 64, 2048
EPS = 1e-5
NCORES = 8
G = 4                         # chunks (= cores per batch)
SQ = S // G                   # 512 own query tokens per core
ND = D // 128                 # 8 feature tiles
NTQ = SQ // 128               # 4 token tiles per chunk
NF = FF // 128                # 16 ff tiles
NHP = H // 2                  # 8 head pairs

_BF = ml_dtypes.bfloat16


def _build_nc():
    nc = bacc.Bacc("TRN2", target_bir_lowering=False, debug=False,
                   num_devices=NCORES)

    xa = nc.dram_tensor("xa", [S, D], BF16, kind="ExternalInput").ap()
    xo = nc.dram_tensor("xo", [SQ, D], F32, kind="ExternalInput").ap()
    wqt = nc.dram_tensor("wqt", [D, D], BF16, kind="ExternalInput").ap()
    wkt = nc.dram_tensor("wkt", [D, D], BF16, kind="ExternalInput").ap()
    wvt = nc.dram_tensor("wvt", [D, D], BF16, kind="ExternalInput").ap()
    wot = nc.dram_tensor("wot", [D, D], BF16, kind="ExternalInput").ap()
    w1t = nc.dram_tensor("w1t", [D, FF], BF16, kind="ExternalInput").ap()
    w2t = nc.dram_tensor("w2t", [FF, D], BF16, kind="ExternalInput").ap()
    bqd = nc.dram_tensor("bq", [128, ND], F32, kind="ExternalInput").ap()
    bkd = nc.dram_tensor("bk", [128, ND], F32, kind="ExternalInput").ap()
    bvd = nc.dram_tensor("bv", [1, D], F32, kind="ExternalInput").ap()
    bod = nc.dram_tensor("bo", [128, ND], F32, kind="ExternalInput").ap()
    b1d = nc.dram_tensor("b1", [128, NF], F32, kind="ExternalInput").ap()
    b2d = nc.dram_tensor("b2", [128, ND], F32, kind="ExternalInput").ap()
    outd = nc.dram_tensor("out", [SQ, D], F32, kind="ExternalOutput").ap()

    with tile.TileContext(nc) as tc, ExitStack() as ctx:
        # ---- whole-kernel pools ----
        const = ctx.enter_context(tc.tile_pool(name="const", bufs=1))
        xrp = ctx.enter_context(tc.tile_pool(name="xrp", bufs=1))
        lns = ctx.enter_context(tc.tile_pool(name="lns", bufs=6))

        eps_sb = const.tile([128, 1], F32, tag="eps")
        nc.vector.memset(eps_sb, EPS)
        bq_sb = const.tile([128, ND], F32, tag="bq")
        nc.gpsimd.dma_start(out=bq_sb, in_=bqd)
        bk_sb = const.tile([128, ND], F32, tag="bk")
        nc.gpsimd.dma_start(out=bk_sb, in_=bkd)
        bo_sb = const.tile([128, ND], F32, tag="bo")
        nc.gpsimd.dma_start(out=bo_sb, in_=bod)
        b1_sb = const.tile([128, NF], F32, tag="b1")
        nc.gpsimd.dma_start(out=b1_sb, in_=b1d)
        b2_sb = const.tile([128, ND], F32, tag="b2")
        nc.gpsimd.dma_start(out=b2_sb, in_=b2d)
        bv_bc = const.tile([128, D], F32, tag="bvb")
        nc.gpsimd.dma_start(out=bv_bc[0:1, :], in_=bvd)
        nc.gpsimd.partition_broadcast(bv_bc, bv_bc[0:1, :])

        # own x (fp32) for the residual -- DMAs deferred (emitted after
        # chunk-0/1 xa loads so they don't delay the critical path)
        xb = [xrp.tile([128, D], F32, tag=f"xb{t}", name=f"xb{t}")
              for t in range(NTQ)]
        x_res = [xrp.tile([128, D], F32, tag=f"xr{t}", name=f"xr{t}")
                 for t in range(NTQ)]

        # o_fm outlives ctxA (consumed by the out-projection) -- pool
        # opened before ctxA's pools to satisfy LIFO pool order
        ctxB = ExitStack()
        ofp = ctxB.enter_context(tc.tile_pool(name="ofp", bufs=1))
        o_fm = [ofp.tile([128, SQ], BF16, tag=f"o{j}", name=f"o{j}")
                for j in range(ND)]

        # ---- attention-scope pools ----
        ctxA = ExitStack()
        xap = ctxA.enter_context(tc.tile_pool(name="xap", bufs=6))
        zp = ctxA.enter_context(tc.tile_pool(name="zp", bufs=2))
        kp = ctxA.enter_context(tc.tile_pool(name="kp", bufs=16))
        vp = ctxA.enter_context(tc.tile_pool(name="vp", bufs=8))
        qp = ctxA.enter_context(tc.tile_pool(name="qp", bufs=1))
        accp = ctxA.enter_context(tc.tile_pool(name="accp", bufs=1))
        stp = ctxA.enter_context(tc.tile_pool(name="stp", bufs=4))
        psA = ctxA.enter_context(tc.tile_pool(name="psA", bufs=2,
                                              space="PSUM"))
        pgp = ctxA.enter_context(tc.tile_pool(name="pgp", bufs=2,
                                              space="PSUM"))
        ppvp = ctxA.enter_context(tc.tile_pool(name="ppvp", bufs=2,
                                               space="PSUM"))
        wkvp = ctxA.enter_context(tc.tile_pool(name="wkvp", bufs=16))
        wqp = ctxA.enter_context(tc.tile_pool(name="wqp", bufs=8))

        wk_sb, wv_sb, wq_sb = [], [], []
        q_fm = [qp.tile([128, SQ], BF16, tag=f"q{d}", name=f"q{d}")
                for d in range(ND)]
        # acc[hp][p, s, q]: bf16 SBUF accumulator (64 feat + 1 den row)
        acc = [accp.tile([DK + 1, 2, SQ], BF16, tag=f"ac{h}",
                         name=f"ac{h}") for h in range(NHP)]

        def ln_chunk(c, zq_c):
            """LN1 over chunk c's 4 token tiles -> feature-major zq_c."""
            mv4 = lns.tile([128, NTQ, 2], F32, tag="mv4")
            xat = []
            for t in range(NTQ):
                xt = xap.tile([128, D], BF16, tag="xa", name=f"xa{c}_{t}")
                nc.gpsimd.dma_start(
                    out=xt, in_=xa[(c * NTQ + t) * 128:
                                   (c * NTQ + t + 1) * 128, :])
                xat.append(xt)
            for t in range(NTQ):
                st = lns.tile([128, 2, 6], F32, tag="st")
                nc.vector.bn_stats(st[:, 0, :], xat[t][:, 0:512])
                nc.vector.bn_stats(st[:, 1, :], xat[t][:, 512:1024])
                nc.vector.bn_aggr(mv4[:, t, :], st)
            sq4 = lns.tile([128, NTQ], F32, tag="sq4")
            nc.scalar.activation(sq4, mv4[:, :, 1], AF.Sqrt,
                                 bias=eps_sb[:, 0:1], scale=1.0)
            rstd4 = lns.tile([128, NTQ], F32, tag="rstd4")
            nc.vector.reciprocal(rstd4, sq4)
            nmr4 = lns.tile([128, NTQ], F32, tag="nmr4")
            nc.vector.scalar_tensor_tensor(nmr4, mv4[:, :, 0], -1.0,
                                           rstd4, op0=OP.mult, op1=OP.mult)
            for t in range(NTQ):
                z_tm = lns.tile([128, D], BF16, tag="ztm", bufs=2)
                nc.vector.tensor_scalar(z_tm, xat[t], rstd4[:, t:t + 1],
                                        nmr4[:, t:t + 1],
                                        op0=OP.mult, op1=OP.add)
                nc.sync.dma_start_transpose(
                    zq_c[:, :, t * 128:(t + 1) * 128], z_tm)

        def k_chain(c, zq_c, j, k_c):
            kt_ = kp.tile([128, SQ], BF16, tag="kc", name=f"k{c}_{j}")
            pk = psA.tile([128, 512], F32, tag="ps")
            for d in range(ND):
                nc.tensor.matmul(pk, wk_sb[d][:, j * 128:(j + 1) * 128],
                                 zq_c[:, d, :],
                                 start=(d == 0), stop=(d == ND - 1))
            nc.vector.tensor_scalar(kt_, pk, bk_sb[:, j:j + 1], None,
                                    op0=OP.add)
            k_c.append(kt_)

        def v_chain(c, zq_c, t, ch, v_c):
            if ch == 0:
                va = vp.tile([128, H, DK + 1], BF16, tag="vc",
                             name=f"v{c}_{t}")
                nc.vector.memset(va[:, :, DK:DK + 1], 1.0)
                v_c.append(va)
            va = v_c[t]
            pv = psA.tile([128, 512], F32, tag="ps")
            for d in range(ND):
                nc.tensor.matmul(
                    pv, zq_c[:, d, t * 128:(t + 1) * 128],
                    wv_sb[d][:, ch * 512:(ch + 1) * 512],
                    start=(d == 0), stop=(d == ND - 1))
            nc.vector.tensor_add(
                va[:, ch * 8:(ch + 1) * 8, 0:DK],
                pv.rearrange("p (h d) -> p h d", h=8),
                bv_bc[:, ch * 512:(ch + 1) * 512].rearrange(
                    "p (h d) -> p h d", h=8))

        def q_chain(j):
            pq = psA.tile([128, 512], F32, tag="ps")
            for d in range(ND):
                nc.tensor.matmul(pq, wq_sb[d][:, j * 128:(j + 1) * 128],
                                 zq[0][:, d, :],
                                 start=(d == 0), stop=(d == ND - 1))
            nc.vector.tensor_scalar(q_fm[j], pq, bq_sb[:, j:j + 1], None,
                                    op0=OP.add)

        # --- preamble: LN1(own) -> K(own) -> Q/V(own) interleaved ---
        zq = [None] * G
        k_ch = [[] for _ in range(G)]
        v_ch = [[] for _ in range(G)]
        for d in range(ND):
            w = wkvp.tile([128, D], BF16, tag="wkv", name=f"wk{d}")
            nc.sync.dma_start(out=w, in_=wkt[d * 128:(d + 1) * 128, :])
            wk_sb.append(w)
        for d in range(ND):
            w = wqp.tile([128, D], BF16, tag="wq", name=f"wq{d}")
            nc.sync.dma_start(out=w, in_=wqt[d * 128:(d + 1) * 128, :])
            wq_sb.append(w)
        zq[0] = zp.tile([128, ND, SQ], BF16, tag="zq", name="zq0")
        ln_chunk(0, zq[0])
        # wv behind chunk-0's z transposes on sync (needed ~10us later)
        for d in range(ND):
            w = wkvp.tile([128, D], BF16, tag="wkv", name=f"wv{d}")
            nc.sync.dma_start(out=w, in_=wvt[d * 128:(d + 1) * 128, :])
            wv_sb.append(w)
        for t in range(NTQ):
            nc.gpsimd.dma_start(out=xb[t], in_=xo[t * 128:(t + 1) * 128, :])
        for j in range(ND):
            k_chain(0, zq[0], j, k_ch[0])
        q_chain(0)
        q_chain(1)

        def normalize_hp(hp):
            # o_fm = acc[0:64] / acc[64] (den row)
            for s in range(2):
                den_f = stp.tile([1, 512], F32, tag="denf", bufs=2)
                nc.vector.tensor_copy(den_f, acc[hp][DK:DK + 1, s, :])
                den_r = stp.tile([1, 512], F32, tag="denr", bufs=2)
                nc.vector.reciprocal_approx_fast(den_r, den_f)
                rb = stp.tile([64, 512], F32, tag="rb", bufs=2)
                nc.gpsimd.partition_broadcast(rb, den_r)
                nc.vector.scalar_tensor_tensor(
                    o_fm[hp][s * 64:(s + 1) * 64, :],
                    acc[hp][0:DK, s, :], 1.0, rb,
                    op0=OP.mult, op1=OP.mult)

        # --- chunk-outer attention with deadline-scheduled filler chains ---
        # Filler units (own V / remaining Q / next-chunk K+V) are emitted
        # into the in-order PE queue just ahead of their first consumer.
        def fillers_for(c, hp):
            out = []
            if c == 0:
                sched = {1: [("q", 2), ("q", 3)],
                         2: [("q", 4), ("q", 5)],
                         3: [("q", 6), ("q", 7), ("v", 0, 0, 1)],
                         4: [("v", 0, 1, 1), ("v", 0, 2, 1)],
                         5: [("v", 0, 3, 1), ("k", 1, 0)],
                         6: [("k", 1, 1), ("v", 1, 0, 0)],
                         7: [("v", 1, 1, 0), ("v", 1, 2, 0),
                             ("v", 1, 3, 0)]}
            elif c < G - 1:
                sched = {0: [("k", c, 2), ("k", c, 3)],
                         1: [("k", c, 4), ("k", c, 5)],
                         2: [("k", c, 6), ("k", c, 7)],
                         3: [("v", c, 0, 1), ("v", c, 1, 1)],
                         4: [("v", c, 2, 1), ("v", c, 3, 1)],
                         5: [("k", c + 1, 0), ("k", c + 1, 1)],
                         6: [("v", c + 1, 0, 0), ("v", c + 1, 1, 0)],
                         7: [("v", c + 1, 2, 0), ("v", c + 1, 3, 0)]}
            else:
                sched = {0: [("k", c, 2), ("k", c, 3)],
                         1: [("k", c, 4), ("k", c, 5)],
                         2: [("k", c, 6), ("k", c, 7)],
                         3: [("v", c, 0, 1), ("v", c, 1, 1)],
                         4: [("v", c, 2, 1), ("v", c, 3, 1)]}
            return sched.get(hp, [])

        def emit_filler(u):
            if u[0] == "q":
                q_chain(u[1])
            elif u[0] == "k":
                k_chain(u[1], zq[u[1]], u[2], k_ch[u[1]])
            else:
                v_chain(u[1], zq[u[1]], u[2], u[3], v_ch[u[1]])

        for c in range(G):
            if c + 1 < G:
                zq[c + 1] = zp.tile([128, ND, SQ], BF16, tag="zq",
                                    name=f"zq{c + 1}")
            for hp in range(NHP):
                if c + 1 < G and hp == 3:
                    ln_chunk(c + 1, zq[c + 1])
                fills = list(fillers_for(c, hp))
                ppv = [ppvp.tile([DK + 1, 512], F32, tag="ppv",
                                 name=f"pp{c}_{hp}_{i}") for i in range(2)]
                prev_st = None
                if c == 0 and hp == 0:
                    for t in range(NTQ):
                        v_chain(0, zq[0], t, 0, v_ch[0])
                for kt in range(NTQ + 1):
                    if kt < NTQ:
                        pg = pgp.tile([128, 2, 512], F32, tag="pg")
                        nc.tensor.matmul(
                            pg[:, 0, :],
                            k_ch[c][hp][0:64, kt * 128:(kt + 1) * 128],
                            q_fm[hp][0:64, :], start=True, stop=True)
                        nc.tensor.matmul(
                            pg[:, 1, :],
                            k_ch[c][hp][64:128, kt * 128:(kt + 1) * 128],
                            q_fm[hp][64:128, :], start=True, stop=True)
                        stg = stp.tile([128, 2, 512], BF16, tag="st")
                        nc.scalar.activation(stg, pg, AF.Exp, bias=0.0,
                                             scale=0.125)
                    if kt > 0:
                        for s in range(2):
                            nc.tensor.matmul(
                                ppv[s],
                                v_ch[c][kt - 1][:, 2 * hp + s, :],
                                prev_st[:, s, :],
                                start=(kt == 1), stop=(kt == NTQ))
                    prev_st = stg
                for u in fills:
                    emit_filler(u)
                # spill/accumulate on DVE (PSUM -> SBUF bf16)
                for s in range(2):
                    if c == 0:
                        nc.vector.tensor_copy(acc[hp][:, s, :], ppv[s])
                    else:
                        nc.vector.tensor_add(acc[hp][:, s, :],
                                             acc[hp][:, s, :], ppv[s])
                if c == G - 1:
                    normalize_hp(hp)

        ctxA.close()

        # --- out-projection + residual (o_fm/wo live) ---
        wop = ctxB.enter_context(tc.tile_pool(name="wop", bufs=8))
        wo_sb = []
        for d in range(ND):
            w = wop.tile([128, D], BF16, tag="wo", name=f"wo{d}")
            nc.gpsimd.dma_start(out=w, in_=wot[d * 128:(d + 1) * 128, :])
            wo_sb.append(w)
        ytp = ctxB.enter_context(tc.tile_pool(name="ytp", bufs=2))
        yp = ctxB.enter_context(tc.tile_pool(name="yp", bufs=1))
        psB = ctxB.enter_context(tc.tile_pool(name="psB", bufs=4,
                                              space="PSUM"))
        y_tm = yp.tile([128, NTQ, D], BF16, tag="ytm", name="y_tm")
        for o in range(ND):
            py = psB.tile([128, 512], F32, tag="psb")
            for j in range(ND):
                nc.tensor.matmul(py, wo_sb[j][:, o * 128:(o + 1) * 128],
                                 o_fm[j], start=(j == 0), stop=(j == ND - 1))
            y_tmp = ytp.tile([128, 512], BF16, tag="yt")
            nc.vector.tensor_scalar(y_tmp, py, bo_sb[:, o:o + 1], None,
                                    op0=OP.add)
            nc.sync.dma_start_transpose(y_tm[:, :, o * 128:(o + 1) * 128],
                                        y_tmp)
        for t in range(NTQ):
            nc.gpsimd.tensor_add(x_res[t], y_tm[:, t, :], xb[t])
        ctxB.close()

        # ---- LN2 + MLP + residual ----
        ctxC = ExitStack()
        z2p = ctxC.enter_context(tc.tile_pool(name="z2p", bufs=1))
        hp_ = ctxC.enter_context(tc.tile_pool(name="hp", bufs=1))
        wB = ctxC.enter_context(tc.tile_pool(name="wB", bufs=9))
        w2p = ctxC.enter_context(tc.tile_pool(name="w2p", bufs=16))
        y2tp = ctxC.enter_context(tc.tile_pool(name="y2tp", bufs=2))
        outp = ctxC.enter_context(tc.tile_pool(name="outp", bufs=1))
        psC = ctxC.enter_context(tc.tile_pool(name="psC", bufs=6,
                                              space="PSUM"))
        z2q = z2p.tile([128, ND, SQ], BF16, tag="z2q", name="z2q")
        h_fm = [hp_.tile([128, SQ], BF16, tag=f"h{f}", name=f"h{f}")
                for f in range(NF)]
        out_tm = [outp.tile([128, D], F32, tag=f"ot{t}", name=f"ot{t}")
                  for t in range(NTQ)]

        w1_sb = []
        for d in range(ND):
            w = wB.tile([128, FF], BF16, tag="wB", name=f"w1_{d}")
            nc.sync.dma_start(out=w, in_=w1t[d * 128:(d + 1) * 128, :])
            w1_sb.append(w)
        w2_sb = []
        for f in range(NF):
            w = w2p.tile([128, D], BF16, tag="w2p", name=f"w2_{f}")
            nc.scalar.dma_start(out=w, in_=w2t[f * 128:(f + 1) * 128, :])
            w2_sb.append(w)

        mv4 = lns.tile([128, NTQ, 2], F32, tag="mv4")
        for t in range(NTQ):
            st = lns.tile([128, 2, 6], F32, tag="st")
            nc.vector.bn_stats(st[:, 0, :], x_res[t][:, 0:512])
            nc.vector.bn_stats(st[:, 1, :], x_res[t][:, 512:1024])
            nc.vector.bn_aggr(mv4[:, t, :], st)
        sq4 = lns.tile([128, NTQ], F32, tag="sq4")
        nc.scalar.activation(sq4, mv4[:, :, 1], AF.Sqrt,
                             bias=eps_sb[:, 0:1], scale=1.0)
        rstd4 = lns.tile([128, NTQ], F32, tag="rstd4")
        nc.vector.reciprocal(rstd4, sq4)
        nmr4 = lns.tile([128, NTQ], F32, tag="nmr4")
        nc.vector.scalar_tensor_tensor(nmr4, mv4[:, :, 0], -1.0, rstd4,
                                       op0=OP.mult, op1=OP.mult)
        for t in range(NTQ):
            z2_tm = lns.tile([128, D], BF16, tag="z2tm", bufs=2)
            nc.vector.tensor_scalar(z2_tm, x_res[t], rstd4[:, t:t + 1],
                                    nmr4[:, t:t + 1],
                                    op0=OP.mult, op1=OP.add)
            nc.sync.dma_start_transpose(
                z2q[:, :, t * 128:(t + 1) * 128], z2_tm)

        for f in range(NF):
            ph = psC.tile([128, 512], F32, tag="psc")
            for d in range(ND):
                nc.tensor.matmul(ph, w1_sb[d][:, f * 128:(f + 1) * 128],
                                 z2q[:, d, :], start=(d == 0),
                                 stop=(d == ND - 1))
            nc.vector.tensor_scalar(h_fm[f], ph, b1_sb[:, f:f + 1], 0.0,
                                    op0=OP.add, op1=OP.max)

        y2_tm = outp.tile([128, NTQ, D], BF16, tag="y2tm", name="y2_tm")
        for o in range(ND):
            p2 = psC.tile([128, 512], F32, tag="psc")
            for f in range(NF):
                nc.tensor.matmul(p2, w2_sb[f][:, o * 128:(o + 1) * 128],
                                 h_fm[f], start=(f == 0), stop=(f == NF - 1))
            y2_tmp = y2tp.tile([128, 512], BF16, tag="y2t")
            nc.vector.tensor_scalar(y2_tmp, p2, b2_sb[:, o:o + 1], None,
                                    op0=OP.add)
            nc.sync.dma_start_transpose(y2_tm[:, :, o * 128:(o + 1) * 128],
                                        y2_tmp)
        for t in range(NTQ):
            nc.vector.tensor_add(out_tm[t], y2_tm[:, t, :], x_res[t])
            nc.sync.dma_start(out=outd[t * 128:(t + 1) * 128, :],
                              in_=out_tm[t])
        ctxC.close()

    nc.compile()
    return nc


_LOCK = threading.Lock()
_NC = None


def _get_nc():
    global _NC
    with _LOCK:
        if _NC is None:
            _NC = _build_nc()
    return _NC


def _prep_inputs(inputs):
    x = np.asarray(inputs["x"], np.float32)
    g1 = np.asarray(inputs["ln1_g"], np.float32)
    b1v = np.asarray(inputs["ln1_b"], np.float32)
    g2 = np.asarray(inputs["ln2_g"], np.float32)
    b2v = np.asarray(inputs["ln2_b"], np.float32)
    wq = np.asarray(inputs["wq"], np.float32)
    wk = np.asarray(inputs["wk"], np.float32)
    wv = np.asarray(inputs["wv"], np.float32)
    wo = np.asarray(inputs["wo"], np.float32)
    w1 = np.asarray(inputs["w1"], np.float32)
    w2 = np.asarray(inputs["w2"], np.float32)

    shared = {
        "wqt": np.ascontiguousarray((g1[:, None] * wq.T)).astype(_BF),
        "wkt": np.ascontiguousarray((g1[:, None] * wk.T)).astype(_BF),
        "wvt": np.ascontiguousarray((g1[:, None] * wv.T)).astype(_BF),
        "wot": np.ascontiguousarray(wo.T).astype(_BF),
        "w1t": np.ascontiguousarray((g2[:, None] * w1.T)).astype(_BF),
        "w2t": np.ascontiguousarray(w2.T).astype(_BF),
        "bq": np.ascontiguousarray(
            (inputs["bq"] + wq @ b1v).astype(np.float32).reshape(ND, 128).T),
        "bk": np.ascontiguousarray(
            (inputs["bk"] + wk @ b1v).astype(np.float32).reshape(ND, 128).T),
        "bv": (inputs["bv"] + wv @ b1v).astype(np.float32).reshape(1, D),
        "bo": np.ascontiguousarray(
            np.asarray(inputs["bo"], np.float32).reshape(ND, 128).T),
        "b1": np.ascontiguousarray(
            (inputs["b1"] + w1 @ b2v).astype(np.float32).reshape(NF, 128).T),
        "b2": np.ascontiguousarray(
            np.asarray(inputs["b2"], np.float32).reshape(ND, 128).T),
    }

    in_maps = []
    for c in range(NCORES):
        b = c // G
        qoff = (c % G) * SQ
        xb = x[b]
        x_perm = np.ascontiguousarray(
            np.concatenate([xb[qoff:qoff + SQ], xb[:qoff], xb[qoff + SQ:]],
                           axis=0))
        m = dict(shared)
        m["xa"] = x_perm.astype(_BF)
        m["xo"] = np.ascontiguousarray(xb[qoff:qoff + SQ])
        in_maps.append(m)
    return in_maps


def _run(inputs, trace=False, tmpdir=None):
    nc = _get_nc()
    in_maps = _prep_inputs(inputs)
    res = run_bass_kernel_spmd(nc, in_maps, core_ids=list(range(NCORES)),
                               trace=trace, tmpdir=tmpdir)
    out = np.empty((B, S, D), np.float32)
    for c in range(NCORES):
        b = c // G
        qoff = (c % G) * SQ
        out[b, qoff:qoff + SQ] = res.results[c]["out"]
    return out, res


def kernel(**inputs):
    out, _ = _run(inputs, trace=False)
    return out
